# revision 3
# baseline (speedup 1.0000x reference)
"""BigBird block-sparse attention TRN2 kernel v2 (8 NeuronCores, SPMD).

Sharding: core c handles batch b=c//2 and head-half hh=c%2 (8 of 16 heads,
feature slice hh*512..+512). All matmul I/O in bf16 (fp32 PSUM accumulate).

Per core, single interleaved pass, q/k/v SBUF-resident (no DRAM roundtrip):
  1. v-pass (streams x once): v = X@Wv.T + bv -> vout DRAM (for host edge PV)
     and, via SBUF->SBUF shift DMAs, into vsh [128, 33, 520]: 64-row-shifted
     key chunks (chunk c = seq 64+128c), chunk 32 = [block63 | block0], with
     a ones column appended per head (col h*65+64) for softmax denominators.
  2. per m-tile mt (2 heads): k.T, q.T feature-major into SBUF [128, 4096]
     bf16 tiles (q pre-scaled by 1/8, biases via DVE tensor_scalar), then
     heads 2mt, 2mt+1:
       - 15 middle strips of 4 query blocks: QK^T transposed (keys on
         partitions, K=64 contraction, odd head at base partition 64),
         exp on ACT (PSUM->SBUF bf16), sliding-window ban by multiplying
         with a {0,1} bf16 mask AFTER exp (DVE 4x mode), PV matmuls against
         vsh chunks; ships numerator+denominator [65, 3840] to host.
       - edge blocks: raw scores for blocks 0/63 (vs all keys) and 1/62
         (vs 6 key blocks, via two-range APs) shipped to host which does
         exp/mask/PV (tiny FLOPs, avoids on-device transposes).
Host: normalizes middle ctx, computes edge PV, reassembles [B, S, HS].
"""
import sys

if "/opt/trn_rl_repo" not in sys.path:
    sys.path.insert(0, "/opt/trn_rl_repo")

import numpy as np
import ml_dtypes

import concourse.bacc as bacc
import concourse.bass as bass
import concourse.tile as tile
from concourse import mybir
from concourse.bass_utils import run_bass_kernel_spmd

F32 = mybir.dt.float32
BF16 = mybir.dt.bfloat16
NPBF16 = ml_dtypes.bfloat16

B, S, H, HS, D, BLK = 4, 4096, 16, 1024, 64, 64
NB = S // BLK            # 64 key/query blocks
HPC = 8                  # heads per core
FPC = HPC * D            # 512 features per core
NKC = HS // 128          # 8 contraction chunks in phase 1
NSEQ = 8                 # phase-1 seq chunks of 512
NMID = 15                # middle strips of 4 blocks (blocks 2..61)
GC = 32                  # index of the [blk63|blk0] global chunk in vsh

_BUILT = None


def _build():
    nc = bacc.Bacc(None, target_bir_lowering=False)

    # ---- parameters ----
    # xt[p, n, kc, s'] = X[n*512+s', kc*128+p]
    xt = nc.declare_dram_parameter("xt", [128, NSEQ, NKC, 512], BF16, False)
    # w*[p, kc, f] = W.T[kc*128+p, f]  (feature slice of this core)
    wq = nc.declare_dram_parameter("wq", [128, NKC, FPC], BF16, False)
    wk = nc.declare_dram_parameter("wk", [128, NKC, FPC], BF16, False)
    wv = nc.declare_dram_parameter("wv", [128, NKC, FPC], BF16, False)
    bqs = nc.declare_dram_parameter("bqs", [128, 4], F32, False)
    bks = nc.declare_dram_parameter("bks", [128, 4], F32, False)
    bvb = nc.declare_dram_parameter("bvb", [FPC], F32, False)
    maskb = nc.declare_dram_parameter("maskb", [128, 3, 256], BF16, False)

    ctxt = nc.declare_dram_parameter("ctxt", [HPC * 65, NMID * 256], BF16, True)
    pe1 = nc.declare_dram_parameter("pe1", [HPC * 128, S], BF16, True)
    pe2 = nc.declare_dram_parameter("pe2", [HPC * 128, 6 * BLK], BF16, True)
    vout = nc.declare_dram_parameter("vout", [S, FPC], BF16, True)

    with tile.TileContext(nc) as tc:
        with tc.tile_pool(name="const", bufs=1) as cp, \
             tc.tile_pool(name="big", bufs=1) as bp, \
             tc.tile_pool(name="x", bufs=2) as xp, \
             tc.tile_pool(name="evac", bufs=3) as ep, \
             tc.tile_pool(name="p2", bufs=2) as p2p, \
             tc.tile_pool(name="pt", bufs=3) as ptp, \
             tc.tile_pool(name="ps1", bufs=2, space="PSUM") as pp1, \
             tc.tile_pool(name="qk", bufs=2, space="PSUM") as qkp, \
             tc.tile_pool(name="sm", bufs=2, space="PSUM") as smp:

            # ---- v-pass constants (the rest loads during the v-pass) ----
            wts = {}
            wts["v"] = cp.tile([128, NKC, FPC], BF16, tag="wv", name="wvt")
            nc.scalar.dma_start(out=wts["v"][:], in_=wv[:])
            bvt = cp.tile([128, FPC], F32, tag="bvt")
            bv_ap = bvb.ap()
            nc.scalar.dma_start(
                out=bvt[:],
                in_=bass.AP(tensor=bv_ap.tensor, offset=bv_ap.offset,
                            ap=[[0, 128]] + bv_ap.ap),
            )

            # vsh: shifted v chunks + ones cols. [128, 33, 520] bf16
            vsh = bp.tile([128, 33, 520], BF16, tag="vsh")
            ones_base = vsh[:, :, 0:1]
            nc.vector.memset(
                bass.AP(tensor=ones_base.tensor, offset=ones_base.offset + 64,
                        ap=[ones_base.ap[0], [520, 33], [65, HPC], [1, 1]]),
                1.0,
            )

            # ---- v-pass ----
            for n in range(NSEQ):
                xtile = xp.tile([128, NKC, 512], BF16, tag="xt")
                nc.sync.dma_start(out=xtile[:], in_=xt[:, n])
                for sm in range(4):
                    m = 4 * n + sm
                    ps = pp1.tile([128, 512], F32, tag="ps1")
                    for kc in range(NKC):
                        nc.tensor.matmul(
                            ps[:],
                            xtile[:, kc, sm * 128:(sm + 1) * 128],
                            wts["v"][:, kc, :],
                            start=(kc == 0), stop=(kc == NKC - 1),
                        )
                    ev = ep.tile([128, 512], BF16, tag="ev")
                    nc.vector.tensor_add(ev[:], ps[:], bvt[:])
                    nc.gpsimd.dma_start(
                        out=vout[m * 128:(m + 1) * 128, :], in_=ev[:],
                    )
                    # shift into vsh: rows 0:64 -> chunk m-1 (or GC) p 64:128,
                    # rows 64:128 -> chunk m (or GC for m=31) p 0:64
                    ca = GC if m == 0 else m - 1
                    dst = vsh[64:128, ca, 0:64]
                    src = ev[0:64, 0:64]
                    nc.scalar.dma_start(
                        out=bass.AP(tensor=dst.tensor, offset=dst.offset,
                                    ap=[dst.ap[0], [65, HPC], [1, 64]]),
                        in_=bass.AP(tensor=src.tensor, offset=src.offset,
                                    ap=[src.ap[0], [64, HPC], [1, 64]]),
                    )
                    cb = GC if m == 31 else m
                    dst = vsh[0:64, cb, 0:64]
                    src = ev[64:128, 0:64]
                    nc.scalar.dma_start(
                        out=bass.AP(tensor=dst.tensor, offset=dst.offset,
                                    ap=[dst.ap[0], [65, HPC], [1, 64]]),
                        in_=bass.AP(tensor=src.tensor, offset=src.offset,
                                    ap=[src.ap[0], [64, HPC], [1, 64]]),
                    )

            # ---- remaining constants (prefetch during v-pass) ----
            for name, w in (("k", wk), ("q", wq)):
                t = cp.tile([128, NKC, FPC], BF16, tag=f"w{name}")
                nc.sync.dma_start(out=t[:], in_=w[:])
                wts[name] = t
            bqt = cp.tile([128, 4], F32, tag="bqt")
            bkt = cp.tile([128, 4], F32, tag="bkt")
            nc.sync.dma_start(out=bqt[:], in_=bqs[:])
            nc.sync.dma_start(out=bkt[:], in_=bks[:])
            mkt = cp.tile([128, 3, 256], BF16, tag="mkt")
            nc.sync.dma_start(out=mkt[:], in_=maskb[:])

            # ---- kq passes + heads (k and q share one x stream) ----
            xpre = {}
            for mt in range(4):
                ktd = bp.tile([128, S], BF16, tag=f"kt{mt}")
                qtd = bp.tile([128, S], BF16, tag=f"qt{mt}")
                for n in range(NSEQ):
                    if (mt, n) in xpre:
                        xtile = xpre.pop((mt, n))
                    else:
                        xtile = xp.tile([128, NKC, 512], BF16, tag="xt")
                        nc.sync.dma_start(out=xtile[:], in_=xt[:, n])
                    for name, dst, bt in (("k", ktd, bkt), ("q", qtd, bqt)):
                        ps = pp1.tile([128, 512], F32, tag="ps1")
                        for kc in range(NKC):
                            nc.tensor.matmul(
                                ps[:],
                                wts[name][:, kc, mt * 128:(mt + 1) * 128],
                                xtile[:, kc, :],
                                start=(kc == 0), stop=(kc == NKC - 1),
                            )
                        if name == "q":
                            nc.vector.tensor_scalar(
                                out=dst[:, n * 512:(n + 1) * 512], in0=ps[:],
                                scalar1=bt[:, mt:mt + 1], scalar2=0.125,
                                op0=mybir.AluOpType.add, op1=mybir.AluOpType.mult,
                            )
                        else:
                            nc.vector.tensor_scalar(
                                out=dst[:, n * 512:(n + 1) * 512], in0=ps[:],
                                scalar1=bt[:, mt:mt + 1], scalar2=None,
                                op0=mybir.AluOpType.add,
                            )
                if mt + 1 < 4:
                    for n in range(2):
                        xtile = xp.tile([128, NKC, 512], BF16, tag="xt",
                                        name=f"xpre{mt}_{n}")
                        nc.sync.dma_start(out=xtile[:], in_=xt[:, n])
                        xpre[(mt + 1, n)] = xtile
                for e in range(2):
                    _head(nc, tc, 2 * mt + e, ktd, qtd, vsh, mkt,
                          qkp, smp, ptp, p2p, ctxt, pe1, pe2)
    nc.compile()
    return nc


def _two_range(t, p0, c0, stride, n_in):
    """AP over cols {c0:c0+n_in} u {c0+stride:+n_in} at partitions p0:p0+64."""
    base = t[p0:p0 + 64, c0:c0 + n_in]
    return bass.AP(tensor=base.tensor, offset=base.offset,
                   ap=[base.ap[0], [stride, 2], [1, n_in]])


def _head(nc, tc, h, kt, qt, vsh, mkt, qkp, smp, ptp, p2p, ctxt, pe1, pe2):
    p0 = 64 * (h % 2)
    # kglob: [blk63 | blk0] key cols for the global group
    kg = p2p.tile([128, 128], BF16, tag="kg")
    nc.vector.tensor_copy(kg[p0:p0 + 64, 0:64], kt[p0:p0 + 64, S - 64:S])
    nc.vector.tensor_copy(kg[p0:p0 + 64, 64:128], kt[p0:p0 + 64, 0:64])
    # edge q blocks (contiguous copies: matmul operands need 1 free dim):
    # qec cols 0:128 = blocks {0, 63} (e1), cols 128:256 = blocks {1, 62} (e2)
    qec = p2p.tile([128, 256], BF16, tag="qec")
    nc.vector.tensor_copy(qec[p0:p0 + 64, 0:64], qt[p0:p0 + 64, 0:64])
    nc.vector.tensor_copy(qec[p0:p0 + 64, 64:128], qt[p0:p0 + 64, S - 64:S])
    nc.vector.tensor_copy(qec[p0:p0 + 64, 128:192], qt[p0:p0 + 64, 64:128])
    nc.vector.tensor_copy(qec[p0:p0 + 64, 192:256],
                          qt[p0:p0 + 64, S - 128:S - 64])

    ctx_acc = p2p.tile([65, NMID * 256], BF16, tag="ctx")

    def strip(s):
        w = 4 * s + 2
        q0 = w * BLK
        sps = qkp.tile([128, 4, 256], F32, tag="qk", name="sps")
        nc.tensor.matmul(sps[:, 0, :], kg[p0:p0 + 64, :],
                         qt[p0:p0 + 64, q0:q0 + 256], start=True, stop=True)
        for c in range(3):
            col = (w - 1 + 2 * c) * BLK
            nc.tensor.matmul(sps[:, 1 + c, :], kt[p0:p0 + 64, col:col + 128],
                             qt[p0:p0 + 64, q0:q0 + 256], start=True, stop=True)
        pt = ptp.tile([128, 4, 256], BF16, tag="pt", name="pt")
        nc.scalar.activation(pt[:], sps[:], mybir.ActivationFunctionType.Exp)
        nc.vector.tensor_mul(pt[:, 1:4, :], pt[:, 1:4, :], mkt[:])
        cps = smp.tile([65, 256], F32, tag="sm", name="cps")
        nc.tensor.matmul(cps[:], vsh[:, GC, h * 65:h * 65 + 65], pt[:, 0, :],
                         start=True, stop=False)
        for c in range(3):
            nc.tensor.matmul(cps[:], vsh[:, 2 * s + c, h * 65:h * 65 + 65],
                             pt[:, 1 + c, :], start=False, stop=(c == 2))
        nc.vector.tensor_copy(ctx_acc[:, s * 256:(s + 1) * 256], cps[:])

    for s in range(8):
        strip(s)
    # edges mid-head: raw scores to host; PE fills ACT-bound strip gaps and
    # the DVE evac burst drains before the next kq-pass needs DVE.
    pev = p2p.tile([128, S], BF16, tag="pev")
    for c in range(8):
        eps = qkp.tile([128, 512], F32, tag="qk", name="eps")
        nc.tensor.matmul(eps[:], qec[p0:p0 + 64, 0:128],
                         kt[p0:p0 + 64, c * 512:(c + 1) * 512],
                         start=True, stop=True)
        nc.vector.tensor_copy(pev[:, c * 512:(c + 1) * 512], eps[:])
    nc.gpsimd.dma_start(out=pe1[h * 128:(h + 1) * 128, :], in_=pev[:])
    # e2: q blocks {1, 62} vs key blocks {0,1,2} u {61,62,63}
    e2ps = qkp.tile([128, 6 * BLK], F32, tag="qk", name="e2ps")
    nc.tensor.matmul(e2ps[:, 0:192], qec[p0:p0 + 64, 128:256],
                     kt[p0:p0 + 64, 0:192], start=True, stop=True)
    nc.tensor.matmul(e2ps[:, 192:384], qec[p0:p0 + 64, 128:256],
                     kt[p0:p0 + 64, S - 192:S], start=True, stop=True)
    e2ev = p2p.tile([128, 6 * BLK], BF16, tag="pe2e")
    nc.vector.tensor_copy(e2ev[:], e2ps[:])
    nc.gpsimd.dma_start(out=pe2[h * 128:(h + 1) * 128, :], in_=e2ev[:])
    for s in range(8, NMID):
        strip(s)
    nc.gpsimd.dma_start(out=ctxt[h * 65:(h + 1) * 65, :], in_=ctx_acc[:])


def _wshuf(W, fs):
    wt = np.asarray(W, np.float32)[fs, :].T  # [HS, FPC]
    return np.ascontiguousarray(
        wt.reshape(NKC, 128, FPC).transpose(1, 0, 2)).astype(NPBF16)


def _host_inputs(hidden, Wq, bq, Wk, bk, Wv, bv, c):
    b, hh = c // 2, c % 2
    fs = slice(hh * FPC, (hh + 1) * FPC)
    X = np.asarray(hidden[b], np.float32)
    xt = np.ascontiguousarray(
        X.reshape(NSEQ, 512, NKC, 128).transpose(3, 0, 2, 1)).astype(NPBF16)
    maskb = np.zeros((128, 3, 256), np.float32)
    for p in range(128):
        for cc in range(3):
            for j in range(4):
                if -1 <= 2 * cc - 1 + (p >= 64) - j <= 1:
                    maskb[p, cc, j * 64:(j + 1) * 64] = 1.0
    return {
        "xt": xt,
        "wq": _wshuf(Wq, fs),
        "wk": _wshuf(Wk, fs),
        "wv": _wshuf(Wv, fs),
        "bqs": np.ascontiguousarray(
            bq[fs].astype(np.float32).reshape(4, 128).T),
        "bks": np.ascontiguousarray(
            bk[fs].astype(np.float32).reshape(4, 128).T),
        "bvb": bv[fs].astype(np.float32),
        "maskb": maskb.astype(NPBF16),
    }


def _host_finish(res_c):
    """Per-core host post-processing -> [S, FPC] output slice."""
    ctxt = np.asarray(res_c["ctxt"], np.float32)
    p1 = np.asarray(res_c["pe1"], np.float32)
    p2 = np.asarray(res_c["pe2"], np.float32)
    v = np.asarray(res_c["vout"], np.float32)  # [S, FPC]
    out = np.empty((S, FPC), np.float32)
    for h in range(HPC):
        vh = v[:, h * 64:(h + 1) * 64]
        # middle blocks 2..61
        num = ctxt[h * 65:h * 65 + 64, :]
        den = ctxt[h * 65 + 64, :]
        out[2 * BLK:62 * BLK, h * 64:(h + 1) * 64] = (num / den).T
        # E1: blocks 0, 63 (full attention); device ships raw scores
        P = np.exp(p1[h * 128:(h + 1) * 128, :])
        C = (P / P.sum(1, keepdims=True)) @ vh
        out[0:BLK, h * 64:(h + 1) * 64] = C[0:64]
        out[S - BLK:S, h * 64:(h + 1) * 64] = C[64:128]
        # E2: blocks 1, 62; key cols = blocks {0,1,2} then {61,62,63}
        P = np.exp(p2[h * 128:(h + 1) * 128, :])
        P[0:64, 192:320] = 0.0    # block 1 bans blocks 61, 62
        P[64:128, 64:192] = 0.0   # block 62 bans blocks 1, 2
        vk = np.concatenate([vh[0:192], vh[(NB - 3) * BLK:]], 0)
        C = (P / P.sum(1, keepdims=True)) @ vk
        out[BLK:2 * BLK, h * 64:(h + 1) * 64] = C[0:64]
        out[62 * BLK:63 * BLK, h * 64:(h + 1) * 64] = C[64:128]
    return out


def _run(inputs, trace=False):
    global _BUILT
    if _BUILT is None:
        _BUILT = _build()
    core_ids = list(range(8))
    in_maps = [_host_inputs(**inputs, c=c) for c in core_ids]
    res = run_bass_kernel_spmd(_BUILT, in_maps, core_ids, trace=trace)
    out = np.empty((B, S, HS), np.float32)
    for c in core_ids:
        b, hh = c // 2, c % 2
        out[b, :, hh * FPC:(hh + 1) * FPC] = _host_finish(res.results[c])
    return out, res


def kernel(hidden_states, Wq, bq, Wk, bk, Wv, bv):
    inputs = dict(hidden=np.asarray(hidden_states), Wq=np.asarray(Wq),
                  bq=np.asarray(bq), Wk=np.asarray(Wk), bk=np.asarray(bk),
                  Wv=np.asarray(Wv), bv=np.asarray(bv))
    out, _ = _run(inputs, trace=False)
    return out
